# revision 71
# baseline (speedup 1.0000x reference)
"""Trainium2 Bass kernel for the CAP loss (camera-aware proxy memory bank).

v4 strategy (8 NeuronCores, SPMD, raw Bass engine blocks):
  Only the intra-camera denominator actually needs the full center bank:
  denom_intra[i] = sum over the 4000 centers sharing camid[i] of
  exp(<f_i, c>/T). Everything else the loss needs (own logit, same-label
  8-block sums, the first-50-hard-negative prefix) touches at most ~67
  centers per sample and is computed exactly on the host in fp32
  (~70 MFLOP of numpy).

  Device work per core (shard = 500 labels x 8 cams, label-major):
  - The host sorts samples by camid into 8 groups and reorders each
    core's center shard residue-major, so group r's samples only ever
    meet the 500 residue-r columns of the shard: 8 independent matmul
    groups of [n_r samples] x [500 centers] x K=2048.
  - Each center column is streamed through the PE exactly once (the
    full-sims design streamed each twice, for 2 sample half-batches):
    64 fp8 DoubleRow matmuls total, ~13.5 us of PE time, under the
    ~20 us it takes to stream the 8.2 MB fp8 shard from HBM. The kernel
    runs at the per-core HBM roofline ("memory" regime).
  - exp comes out of PSUM on the scalar engine with a constant scale
    (feats are pre-normalized on the host, so the exp scale folds the
    temperature and both fp8 quantization scales into one immediate),
    and a single [n_r, 500] row-sum per group on the vector engine
    yields the per-core denom_intra partial: a [128, 8] f32 output.
  - The PE clock (HAM gate) is warmed by dummy matmuls on a zeroed tile
    that depend only on a DVE memset, and 3 pacing dummies after each
    group keep the PE busy while DMA (the binding resource) delivers
    the next slab, so the clock never re-throttles.
  - Slabs are split into k-halves across two DMA queues (sync + scalar
    HW-DGE rings); each group's first matmuls start on half-arrived data.

Raw Bass (nc.Block) is used instead of the Tile framework: the installed
walrus rejects two raw-ISA instructions Tile's exit barrier emits
(EVENT_SEMAPHORE_RANGE_CLEAR, multi-wait DRAIN) and InstTensorTensorReduce.
"""

import numpy as np
import ml_dtypes
from contextlib import ExitStack

import concourse.bass as bass
from concourse import mybir
from concourse.bass_utils import run_bass_kernel_spmd

# problem constants (hardcoded per harness contract)
N, D, M = 256, 2048, 32000
L, C = 4000, 8
T = 0.07
HARD_NEG_K = 50
LAMDA = 0.5
NCORES = 8
SHARD = M // NCORES          # 4000 centers per core
LBL_SHARD = SHARD // C       # 500 labels per core
KT = D // 128                # 16 k-tiles
DK = KT // 2                 # 8 DoubleRow k-steps (256 rows each)
W = LBL_SHARD                # 500 centers per (core, cam) group
WS = 512                     # slab row stride (DoubleRow needs step%16==0)
NW = 14                      # HAM warmup matmuls (no data dependency)
NPACE = 3                    # pacing dummies per group (PE waits on DMA)

AF = 64.0                    # host fp8 scale for normalized feats
AC = 32.0                    # host fp8 scale for (unit-norm) centers
S_EXP = 1.0 / (T * AF * AC)  # constant exp scale: sims = psum * S_EXP

F32 = mybir.dt.float32
BF16 = mybir.dt.bfloat16
FP8 = mybir.dt.float8e4
ADD = mybir.AluOpType.add
AX = mybir.AxisListType.X
EXP = mybir.ActivationFunctionType.Exp
DROW = mybir.MatmulPerfMode.DoubleRow


W7 = W // 2                  # 250: the last group runs as two column halves
W7S = 256                    # their DRAM/slab stride (contiguous DMA rows)


def _build_program(counts) -> bass.Bass:
    starts = np.concatenate([[0], np.cumsum(counts)]).astype(int)
    assert starts[-1] == N
    assert max(counts) <= 128, "a cam group must fit one PSUM partition dim"
    # an empty cam group still runs one pad row (host ignores its column)
    groups = [(min(int(starts[r]), N - 1), max(int(counts[r]), 1))
              for r in range(C)]

    nc = bass.Bass()
    # cG is padded to the 512 slab stride so every DMA row is a contiguous
    # 4 KB line (500-byte strided rows run the DMA engines at ~65%).
    # Group 7 ships as two 250-column halves so only half a group's matmul
    # + a half-width exp ride the kernel tail.
    cG = nc.dram_tensor("cG", [C - 1, 128, KT, WS], FP8, kind="ExternalInput")
    cG7 = nc.dram_tensor("cG7", [2, 128, KT, W7S], FP8, kind="ExternalInput")
    fT = nc.dram_tensor("fT", [128, KT, N], FP8, kind="ExternalInput")
    di_out = nc.dram_tensor("DI_out", [128, C + 1], F32,
                            kind="ExternalOutput")

    with ExitStack() as ctx:
        e = ctx.enter_context

        ft_sb = e(nc.sbuf_tensor("ft_sb", [128, KT, N], FP8))
        slabs = [e(nc.sbuf_tensor(f"slab{g}", [128, KT, WS], FP8))
                 for g in range(C)]
        eg = e(nc.sbuf_tensor("eg", [128, C, W], BF16))
        di = e(nc.sbuf_tensor("di", [128, C + 1], F32))
        wm = e(nc.sbuf_tensor("wm", [128, WS], FP8))

        pg = [e(nc.psum_tensor(f"pg{g}", [128, W], F32)) for g in range(C)]

        sem_ftl = e(nc.semaphore("sem_ftl"))
        sem_fth = e(nc.semaphore("sem_fth"))
        sem_slo = [e(nc.semaphore(f"sem_slo{g}")) for g in range(C)]
        sem_shi = [e(nc.semaphore(f"sem_shi{g}")) for g in range(C)]
        sem_pe = e(nc.semaphore("sem_pe"))
        sem_act = e(nc.semaphore("sem_act"))
        c_ws = e(nc.semaphore("c_ws"))     # warmup tile memset done
        c_warm = e(nc.semaphore("c_warm"))
        sem_od = e(nc.semaphore("sem_od"))

        block = e(nc.Block(no_gpsimd_drain=True))

        @block.sync
        def _(sync):
            # low k-halves of feats + every slab on the sync ring; high
            # halves ride the scalar ring concurrently, so each group's
            # first matmuls can start on half-arrived data
            sync.dma_start(out=ft_sb[:, 0:8, :],
                           in_=fT[:, 0:8, :]).then_inc(sem_ftl, 16)
            for g in range(C - 1):
                sync.dma_start(out=slabs[g][:, 0:8, :],
                               in_=cG[g, :, 0:8, :]).then_inc(sem_slo[g], 16)
            # group 7's two column halves (in-queue order fixes sem counts)
            for h in range(2):
                sync.dma_start(
                    out=slabs[7][:, 0:8, h * W7S:(h + 1) * W7S],
                    in_=cG7[h, :, 0:8, :]).then_inc(sem_slo[7], 16)
            sync.wait_ge(sem_od, 16)

        @block.tensor
        def _(tensor):
            # dummy matmuls on a zeroed tile: warms the PE clock gate (HAM)
            # while the first slabs are still in flight
            tensor.wait_ge(c_ws, 1)
            last = None
            for _w in range(NW):
                last = tensor.matmul(pg[C - 1][:, :], wm[:, 0:128],
                                     wm[:, 0:W], start=True, stop=True)
            last.then_inc(c_warm, 1)
            # sub-groups: (psum cols, slab, slab col offset, width, sem thr)
            subs = [(g, 0, 0, W, 16) for g in range(C - 1)] + \
                   [(7, 0, 0, W7, 16), (7, W7, W7S, W7, 32)]
            for i, (g, po, so, w, thr) in enumerate(subs):
                s0, ng = groups[g]
                if g == C - 1 and thr == 16:
                    # warmup dummies wrote this psum bank (WAW ordering)
                    tensor.wait_ge(c_warm, 1)
                last = None
                for dk in range(DK):
                    if dk == 0:
                        if i == 0:
                            tensor.wait_ge(sem_ftl, 16)
                        tensor.wait_ge(sem_slo[g], thr)
                    if dk == DK // 2:
                        if i == 0:
                            tensor.wait_ge(sem_fth, 16)
                        tensor.wait_ge(sem_shi[g], thr)
                    last = tensor.matmul(
                        pg[g][0:ng, po:po + w],
                        ft_sb[:, 2 * dk:2 * dk + 2, s0:s0 + ng],
                        slabs[g][:, 2 * dk:2 * dk + 2, so:so + w],
                        start=(dk == 0), stop=(dk == DK - 1),
                        perf_mode=DROW)
                last.then_inc(sem_pe, 1)
                if i < len(subs) - 1:
                    # pacing dummies: keep the PE busy (HAM at full clock)
                    # while DMA, the binding resource, delivers the next
                    # sub-group. They write a bank whose next real use is a
                    # start=True reset; wrapped targets wait for that
                    # bank's exp to have consumed it first
                    tgt = (g + 2) % C
                    if tgt < g:
                        tensor.wait_ge(sem_act, tgt + 1)
                    for _p in range(NPACE):
                        tensor.matmul(pg[tgt][:, 0:W], wm[:, 0:128],
                                      wm[:, 0:W], start=True, stop=True)

        @block.scalar
        def _(scalar):
            # high k-halves ride the ACT engine's own HW-DGE ring (a second
            # DMA queue), in parallel with the sync ring's low halves
            scalar.dma_start(out=ft_sb[:, 8:16, :],
                             in_=fT[:, 8:16, :]).then_inc(sem_fth, 16)
            for g in range(C - 1):
                scalar.dma_start(out=slabs[g][:, 8:16, :],
                                 in_=cG[g, :, 8:16, :]).then_inc(
                    sem_shi[g], 16)
            for h in range(2):
                scalar.dma_start(
                    out=slabs[7][:, 8:16, h * W7S:(h + 1) * W7S],
                    in_=cG7[h, :, 8:16, :]).then_inc(sem_shi[7], 16)
            # exp stream straight out of PSUM with a constant scale; the
            # activation accumulator yields the row-sum (= the denom_intra
            # partial) in the same instruction, so no reduce is needed.
            # Group 7's halves land in di columns 7 and 8 (host adds them)
            esubs = [(g, 0, W, g) for g in range(C - 1)] + \
                    [(7, 0, W7, 7), (7, W7, W7, C)]
            for i, (g, po, w, dcol) in enumerate(esubs):
                ng = groups[g][1]
                scalar.wait_ge(sem_pe, i + 1)
                scalar.activation(
                    out=eg[0:ng, g, po:po + w],
                    in_=pg[g][0:ng, po:po + w],
                    func=EXP, scale=S_EXP,
                    accum_out=di[0:ng, dcol:dcol + 1]).then_inc(sem_act, 1)
            # denom-intra partials out (4.5 KB, single transfer). The wait
            # is required even on the same engine: dma_start only enqueues,
            # so without it the transfer could read di before the last
            # accumulator read has written it
            scalar.wait_ge(sem_act, C + 1)
            scalar.dma_start(out=di_out[:, :],
                             in_=di[:, :]).then_inc(sem_od, 16)

        @block.vector
        def _(vector):
            vector.memset(wm[:, :], 0.0).then_inc(c_ws, 1)

    return nc


_PROGRAM_CACHE: dict[tuple, bass.Bass] = {}


def _program(counts) -> bass.Bass:
    key = tuple(int(x) for x in counts)
    if key not in _PROGRAM_CACHE:
        _PROGRAM_CACHE[key] = _build_program(key)
    return _PROGRAM_CACHE[key]


def _sort_by_cam(camids):
    order = np.argsort(camids, kind="stable")
    counts = np.bincount(camids, minlength=C)
    return order, counts


def _make_in_maps(feats, centers, camids):
    """feats: [256, 2048] f32, centers: [32000, 2048] f32 (unit-norm rows)."""
    fp8 = ml_dtypes.float8_e4m3
    order, _counts = _sort_by_cam(camids)
    f = feats / np.linalg.norm(feats, axis=1, keepdims=True)
    fq = (f[order] * AF).astype(fp8)                   # [256, 2048], cam-major
    fT_t = np.ascontiguousarray(fq.T)                  # [2048, 256]
    fT_t = np.ascontiguousarray(
        fT_t.reshape(KT, 128, N).transpose(1, 0, 2))   # [128, 16, 256]
    cq = (centers * AC).astype(fp8)                    # [32000, 2048] fp8

    in_maps = []
    for c in range(NCORES):
        sub = cq[c * SHARD:(c + 1) * SHARD]            # [4000, 2048]
        # residue-major: group r gets the shard's cam-r columns
        rm = sub.reshape(LBL_SHARD, C, D).transpose(1, 2, 0)  # [8, 2048, 500]
        rm = np.ascontiguousarray(rm)
        tiles = rm.reshape(C, KT, 128, W).transpose(0, 2, 1, 3)
        cg = np.zeros((C - 1, 128, KT, WS), fp8)       # padded to the stride
        cg[:, :, :, 0:W] = tiles[0:C - 1]
        cg7 = np.zeros((2, 128, KT, W7S), fp8)         # group 7 column halves
        cg7[0, :, :, 0:W7] = tiles[C - 1][:, :, 0:W7]
        cg7[1, :, :, 0:W7] = tiles[C - 1][:, :, W7:W]
        in_maps.append({"cG": cg, "cG7": cg7, "fT": fT_t})
    return in_maps


def _host_tail(results, feats, centers, labels, camids, epoch):
    n = labels.shape[0]
    order, counts = _sort_by_cam(camids)
    starts = np.concatenate([[0], np.cumsum(counts)]).astype(int)

    # device part: per-core denom_intra partials, [128, 9] each (group 7
    # arrives as two half-sums in columns 7 and 8)
    DI9 = np.zeros((128, C + 1), np.float32)
    for r in results:
        DI9 += r["DI_out"]
    DI = DI9[:, 0:C].copy()
    DI[:, C - 1] += DI9[:, C]
    di_sorted = np.empty(n, np.float32)
    for g in range(C):
        s0, ng = int(starts[g]), int(counts[g])
        di_sorted[s0:s0 + ng] = DI[0:ng, g]
    denom_intra = np.empty(n, np.float32)
    denom_intra[order] = di_sorted

    # host part (exact fp32): own logit, same-label block sums, hard-neg
    # prefix over the first 50/58 bank columns
    f = feats / np.linalg.norm(feats, axis=1, keepdims=True)
    own_idx = labels * C + camids
    own = np.einsum("nd,nd->n", f,
                    centers[own_idx]).astype(np.float32) / T
    cb = centers[(labels * C)[:, None] + np.arange(C)[None, :]]  # [n, 8, d]
    B = np.exp(np.einsum("nd,ncd->nc", f, cb) / T).sum(axis=1)
    E58 = np.exp((f @ centers[0:HARD_NEG_K + C].T) / T)          # [n, 58]
    p50 = E58[:, 0:HARD_NEG_K].sum(axis=1)
    p58 = E58.sum(axis=1)
    hard = np.where(labels <= 6, p58 - B, p50)
    denom_inter = (B + hard).astype(np.float32)

    loss_i = own - np.log(denom_intra)
    loss_j = own - np.log(denom_inter)

    cam_sums = np.zeros(C, np.float32)
    cam_cnts = np.zeros(C, np.float32)
    np.add.at(cam_sums, camids, loss_i)
    np.add.at(cam_cnts, camids, 1.0)
    loss_intra = -np.sum(
        np.where(cam_cnts > 0, cam_sums / np.maximum(cam_cnts, 1.0), 0.0),
        dtype=np.float32)

    lbl_sums = np.zeros(L, np.float32)
    lbl_cnts = np.zeros(L, np.float32)
    np.add.at(lbl_sums, labels, loss_j)
    np.add.at(lbl_cnts, labels, 1.0)
    loss_inter = -np.sum(
        np.where(lbl_cnts > 0, lbl_sums / np.maximum(lbl_cnts, 1.0), 0.0),
        dtype=np.float32)

    if int(epoch) < 5:
        return np.float32(loss_intra)
    return np.stack([loss_intra, LAMDA * loss_inter]).astype(np.float32)


def kernel(feats, centers, labels, camids, epoch):
    feats = np.ascontiguousarray(np.asarray(feats, dtype=np.float32))
    centers = np.ascontiguousarray(np.asarray(centers, dtype=np.float32))
    labels = np.asarray(labels).astype(np.int64)
    camids = np.asarray(camids).astype(np.int64)

    _order, counts = _sort_by_cam(camids)
    in_maps = _make_in_maps(feats, centers, camids)
    res = run_bass_kernel_spmd(_program(counts), in_maps,
                               list(range(NCORES))).results
    return _host_tail(res, feats, centers, labels, camids, epoch)


# revision 78
# speedup vs baseline: 1.0164x; 1.0164x over previous
"""Trainium2 Bass kernel for the CAP loss (camera-aware proxy memory bank).

v4 strategy (8 NeuronCores, SPMD, raw Bass engine blocks):
  Only the intra-camera denominator actually needs the full center bank:
  denom_intra[i] = sum over the 4000 centers sharing camid[i] of
  exp(<f_i, c>/T). Everything else the loss needs (own logit, same-label
  8-block sums, the first-50-hard-negative prefix) touches at most ~67
  centers per sample and is computed exactly on the host in fp32
  (~70 MFLOP of numpy).

  Device work per core (shard = 500 labels x 8 cams, label-major):
  - The host sorts samples by camid into 8 groups and reorders each
    core's center shard residue-major, so group r's samples only ever
    meet the 500 residue-r columns of the shard: 8 independent matmul
    groups of [n_r samples] x [500 centers] x K=2048.
  - Each center column is streamed through the PE exactly once (the
    full-sims design streamed each twice, for 2 sample half-batches):
    64 fp8 DoubleRow matmuls total, ~13.5 us of PE time, under the
    ~20 us it takes to stream the 8.2 MB fp8 shard from HBM. The kernel
    runs at the per-core HBM roofline ("memory" regime).
  - exp comes out of PSUM on the scalar engine with a constant scale
    (feats are pre-normalized on the host, so the exp scale folds the
    temperature and both fp8 quantization scales into one immediate),
    and a single [n_r, 500] row-sum per group on the vector engine
    yields the per-core denom_intra partial: a [128, 8] f32 output.
  - The PE clock (HAM gate) is warmed by dummy matmuls on a zeroed tile
    that depend only on a DVE memset, and 3 pacing dummies after each
    group keep the PE busy while DMA (the binding resource) delivers
    the next slab, so the clock never re-throttles.
  - Slabs are split into k-halves across two DMA queues (sync + scalar
    HW-DGE rings); each group's first matmuls start on half-arrived data.

Raw Bass (nc.Block) is used instead of the Tile framework: the installed
walrus rejects two raw-ISA instructions Tile's exit barrier emits
(EVENT_SEMAPHORE_RANGE_CLEAR, multi-wait DRAIN) and InstTensorTensorReduce.
"""

import numpy as np
import ml_dtypes
from contextlib import ExitStack

import concourse.bass as bass
from concourse import mybir
from concourse.bass_utils import run_bass_kernel_spmd

# problem constants (hardcoded per harness contract)
N, D, M = 256, 2048, 32000
L, C = 4000, 8
T = 0.07
HARD_NEG_K = 50
LAMDA = 0.5
NCORES = 8
SHARD = M // NCORES          # 4000 centers per core
LBL_SHARD = SHARD // C       # 500 labels per core
KT = D // 128                # 16 k-tiles
DK = KT // 2                 # 8 DoubleRow k-steps (256 rows each)
W = LBL_SHARD                # 500 centers per (core, cam) group
WS = 512                     # slab row stride (DoubleRow needs step%16==0)
NW = 14                      # HAM warmup matmuls (no data dependency)
NPACE = 3                    # pacing dummies per group (PE waits on DMA)

AF = 64.0                    # host fp8 scale for normalized feats
AC = 32.0                    # host fp8 scale for (unit-norm) centers
S_EXP = 1.0 / (T * AF * AC)  # constant exp scale: sims = psum * S_EXP

F32 = mybir.dt.float32
BF16 = mybir.dt.bfloat16
FP8 = mybir.dt.float8e4
ADD = mybir.AluOpType.add
AX = mybir.AxisListType.X
EXP = mybir.ActivationFunctionType.Exp
DROW = mybir.MatmulPerfMode.DoubleRow


W7 = W // 2                  # 250: the last group runs as two column halves
W7S = 256                    # their DRAM/slab stride (contiguous DMA rows)


def _build_program(counts) -> bass.Bass:
    starts = np.concatenate([[0], np.cumsum(counts)]).astype(int)
    assert starts[-1] == N
    assert max(counts) <= 128, "a cam group must fit one PSUM partition dim"
    # an empty cam group still runs one pad row (host ignores its column)
    groups = [(min(int(starts[r]), N - 1), max(int(counts[r]), 1))
              for r in range(C)]

    nc = bass.Bass()
    # cG is padded to the 512 slab stride so every DMA row is a contiguous
    # 4 KB line (500-byte strided rows run the DMA engines at ~65%).
    # Group 7 ships as two 250-column halves so only half a group's matmul
    # + a half-width exp ride the kernel tail.
    cG = nc.dram_tensor("cG", [C - 1, 128, KT, WS], FP8, kind="ExternalInput")
    cG7 = nc.dram_tensor("cG7", [2, 128, KT, W7S], FP8, kind="ExternalInput")
    fT = nc.dram_tensor("fT", [128, KT, N], FP8, kind="ExternalInput")
    di_out = nc.dram_tensor("DI_out", [128, C + 1], F32,
                            kind="ExternalOutput")

    with ExitStack() as ctx:
        e = ctx.enter_context

        ft_sb = e(nc.sbuf_tensor("ft_sb", [128, KT, N], FP8))
        slabs = [e(nc.sbuf_tensor(f"slab{g}", [128, KT, WS], FP8))
                 for g in range(C)]
        slab7b = e(nc.sbuf_tensor("slab7b", [128, KT, W7S], FP8))
        eg = e(nc.sbuf_tensor("eg", [128, C, W], BF16))
        di = e(nc.sbuf_tensor("di", [128, C + 1], F32))
        wm = e(nc.sbuf_tensor("wm", [128, WS], FP8))

        pg = [e(nc.psum_tensor(f"pg{g}", [128, W], F32)) for g in range(C)]

        sem_ftl = e(nc.semaphore("sem_ftl"))
        sem_fth = e(nc.semaphore("sem_fth"))
        sem_slo = [e(nc.semaphore(f"sem_slo{g}")) for g in range(C)]
        sem_shi = [e(nc.semaphore(f"sem_shi{g}")) for g in range(C)]
        sem_7bl = e(nc.semaphore("sem_7bl"))
        sem_7bh = e(nc.semaphore("sem_7bh"))
        sem_pe = e(nc.semaphore("sem_pe"))
        sem_act = e(nc.semaphore("sem_act"))
        c_ws = e(nc.semaphore("c_ws"))     # warmup tile memset done
        c_warm = e(nc.semaphore("c_warm"))
        sem_od = e(nc.semaphore("sem_od"))

        block = e(nc.Block(no_gpsimd_drain=True))

        @block.sync
        def _(sync):
            # low k-halves of feats + every slab on the sync ring; high
            # halves ride the scalar ring concurrently, so each group's
            # first matmuls can start on half-arrived data
            sync.dma_start(out=ft_sb[:, 0:8, :],
                           in_=fT[:, 0:8, :]).then_inc(sem_ftl, 16)
            for g in range(C - 1):
                sync.dma_start(out=slabs[g][:, 0:8, :],
                               in_=cG[g, :, 0:8, :]).then_inc(sem_slo[g], 16)
            # group 7's two column halves (in-queue order fixes sem counts)
            sync.dma_start(out=slabs[7][:, 0:8, 0:W7S],
                           in_=cG7[0, :, 0:8, :]).then_inc(sem_slo[7], 16)
            sync.dma_start(out=slab7b[:, 0:8, :],
                           in_=cG7[1, :, 0:8, :]).then_inc(sem_7bl, 16)
            sync.wait_ge(sem_od, 16)

        @block.tensor
        def _(tensor):
            # dummy matmuls on a zeroed tile: warms the PE clock gate (HAM)
            # while the first slabs are still in flight
            tensor.wait_ge(c_ws, 1)
            last = None
            for _w in range(NW):
                last = tensor.matmul(pg[C - 1][:, :], wm[:, 0:128],
                                     wm[:, 0:W], start=True, stop=True)
            last.then_inc(c_warm, 1)
            # sub-groups: (psum cols, slab, slab col offset, width, sem thr)
            subs = [(g, 0, 0, W, 16) for g in range(C - 1)] + \
                   [(7, 0, 0, W7, 16), (7, W7, W7S, W7, 32)]
            for i, (g, po, so, w, thr) in enumerate(subs):
                s0, ng = groups[g]
                if g == C - 1 and thr == 16:
                    # warmup dummies wrote this psum bank (WAW ordering)
                    tensor.wait_ge(c_warm, 1)
                last = None
                is7b = (g == C - 1 and thr == 32)
                for dk in range(DK):
                    if dk == 0:
                        if i == 0:
                            tensor.wait_ge(sem_ftl, 16)
                        if is7b:
                            tensor.wait_ge(sem_7bl, 16)
                        else:
                            tensor.wait_ge(sem_slo[g], 16)
                    if dk == DK // 2:
                        if i == 0:
                            tensor.wait_ge(sem_fth, 16)
                        if is7b:
                            tensor.wait_ge(sem_7bh, 16)
                        else:
                            tensor.wait_ge(sem_shi[g], 16)
                    rhs_t = slab7b if is7b else slabs[g]
                    roff = 0 if is7b else so
                    last = tensor.matmul(
                        pg[g][0:ng, po:po + w],
                        ft_sb[:, 2 * dk:2 * dk + 2, s0:s0 + ng],
                        rhs_t[:, 2 * dk:2 * dk + 2, roff:roff + w],
                        start=(dk == 0), stop=(dk == DK - 1),
                        perf_mode=DROW)
                last.then_inc(sem_pe, 1)
                if i < len(subs) - 1:
                    # pacing dummies: keep the PE busy (HAM at full clock)
                    # while DMA, the binding resource, delivers the next
                    # sub-group. They write a bank whose next real use is a
                    # start=True reset; wrapped targets wait for that
                    # bank's exp to have consumed it first
                    tgt = (g + 2) % C
                    if tgt < g:
                        tensor.wait_ge(sem_act, tgt + 1)
                    for _p in range(NPACE):
                        tensor.matmul(pg[tgt][:, 0:W], wm[:, 0:128],
                                      wm[:, 0:W], start=True, stop=True)

        @block.scalar
        def _(scalar):
            # high k-halves ride the ACT engine's own HW-DGE ring (a second
            # DMA queue), in parallel with the sync ring's low halves
            scalar.dma_start(out=ft_sb[:, 8:16, :],
                             in_=fT[:, 8:16, :]).then_inc(sem_fth, 16)
            for g in range(C - 1):
                scalar.dma_start(out=slabs[g][:, 8:16, :],
                                 in_=cG[g, :, 8:16, :]).then_inc(
                    sem_shi[g], 16)
            scalar.dma_start(out=slabs[7][:, 8:16, 0:W7S],
                             in_=cG7[0, :, 8:16, :]).then_inc(sem_shi[7], 16)
            scalar.dma_start(out=slab7b[:, 8:16, :],
                             in_=cG7[1, :, 8:16, :]).then_inc(sem_7bh, 16)
            # exp stream straight out of PSUM with a constant scale; the
            # activation accumulator yields the row-sum (= the denom_intra
            # partial) in the same instruction, so no reduce is needed.
            # Group 7's halves land in di columns 7 and 8 (host adds them)
            esubs = [(g, 0, W, g) for g in range(C - 1)] + \
                    [(7, 0, W7, 7), (7, W7, W7, C)]
            scalar.wait_ge(c_ws, 2)
            for i, (g, po, w, dcol) in enumerate(esubs):
                ng = groups[g][1]
                scalar.wait_ge(sem_pe, i + 1)
                scalar.activation(
                    out=eg[0:ng, g, po:po + w],
                    in_=pg[g][0:ng, po:po + w],
                    func=EXP, scale=S_EXP,
                    accum_out=di[0:ng, dcol:dcol + 1]).then_inc(sem_act, 1)
            # denom-intra partials out (4.5 KB, single transfer). The wait
            # is required even on the same engine: dma_start only enqueues,
            # so without it the transfer could read di before the last
            # accumulator read has written it
            scalar.wait_ge(sem_act, C + 1)
            scalar.dma_start(out=di_out[:, :],
                             in_=di[:, :]).then_inc(sem_od, 16)

        @block.vector
        def _(vector):
            vector.memset(wm[:, :], 0.0).then_inc(c_ws, 1)
            vector.memset(di[:, :], 0.0).then_inc(c_ws, 1)

    return nc


_PROGRAM_CACHE: dict[tuple, bass.Bass] = {}


def _program(counts) -> bass.Bass:
    key = tuple(int(x) for x in counts)
    if key not in _PROGRAM_CACHE:
        _PROGRAM_CACHE[key] = _build_program(key)
    return _PROGRAM_CACHE[key]


def _sort_by_cam(camids):
    order = np.argsort(camids, kind="stable")
    counts = np.bincount(camids, minlength=C)
    return order, counts


def _make_in_maps(feats, centers, camids):
    """feats: [256, 2048] f32, centers: [32000, 2048] f32 (unit-norm rows)."""
    fp8 = ml_dtypes.float8_e4m3
    order, _counts = _sort_by_cam(camids)
    f = feats / np.linalg.norm(feats, axis=1, keepdims=True)
    fq = (f[order] * AF).astype(fp8)                   # [256, 2048], cam-major
    fT_t = np.ascontiguousarray(fq.T)                  # [2048, 256]
    fT_t = np.ascontiguousarray(
        fT_t.reshape(KT, 128, N).transpose(1, 0, 2))   # [128, 16, 256]
    cq = (centers * AC).astype(fp8)                    # [32000, 2048] fp8

    in_maps = []
    for c in range(NCORES):
        sub = cq[c * SHARD:(c + 1) * SHARD]            # [4000, 2048]
        # residue-major: group r gets the shard's cam-r columns
        rm = sub.reshape(LBL_SHARD, C, D).transpose(1, 2, 0)  # [8, 2048, 500]
        rm = np.ascontiguousarray(rm)
        tiles = rm.reshape(C, KT, 128, W).transpose(0, 2, 1, 3)
        cg = np.zeros((C - 1, 128, KT, WS), fp8)       # padded to the stride
        cg[:, :, :, 0:W] = tiles[0:C - 1]
        cg7 = np.zeros((2, 128, KT, W7S), fp8)         # group 7 column halves
        cg7[0, :, :, 0:W7] = tiles[C - 1][:, :, 0:W7]
        cg7[1, :, :, 0:W7] = tiles[C - 1][:, :, W7:W]
        in_maps.append({"cG": cg, "cG7": cg7, "fT": fT_t})
    return in_maps


def _host_tail(results, feats, centers, labels, camids, epoch):
    n = labels.shape[0]
    order, counts = _sort_by_cam(camids)
    starts = np.concatenate([[0], np.cumsum(counts)]).astype(int)

    # device part: per-core denom_intra partials, [128, 9] each (group 7
    # arrives as two half-sums in columns 7 and 8)
    DI9 = np.zeros((128, C + 1), np.float32)
    for r in results:
        DI9 += r["DI_out"]
    DI = DI9[:, 0:C].copy()
    DI[:, C - 1] += DI9[:, C]
    di_sorted = np.empty(n, np.float32)
    for g in range(C):
        s0, ng = int(starts[g]), int(counts[g])
        di_sorted[s0:s0 + ng] = DI[0:ng, g]
    denom_intra = np.empty(n, np.float32)
    denom_intra[order] = di_sorted

    # host part (exact fp32): own logit, same-label block sums, hard-neg
    # prefix over the first 50/58 bank columns
    f = feats / np.linalg.norm(feats, axis=1, keepdims=True)
    own_idx = labels * C + camids
    own = np.einsum("nd,nd->n", f,
                    centers[own_idx]).astype(np.float32) / T
    cb = centers[(labels * C)[:, None] + np.arange(C)[None, :]]  # [n, 8, d]
    B = np.exp(np.einsum("nd,ncd->nc", f, cb) / T).sum(axis=1)
    E58 = np.exp((f @ centers[0:HARD_NEG_K + C].T) / T)          # [n, 58]
    p50 = E58[:, 0:HARD_NEG_K].sum(axis=1)
    p58 = E58.sum(axis=1)
    hard = np.where(labels <= 6, p58 - B, p50)
    denom_inter = (B + hard).astype(np.float32)

    loss_i = own - np.log(denom_intra)
    loss_j = own - np.log(denom_inter)

    cam_sums = np.zeros(C, np.float32)
    cam_cnts = np.zeros(C, np.float32)
    np.add.at(cam_sums, camids, loss_i)
    np.add.at(cam_cnts, camids, 1.0)
    loss_intra = -np.sum(
        np.where(cam_cnts > 0, cam_sums / np.maximum(cam_cnts, 1.0), 0.0),
        dtype=np.float32)

    lbl_sums = np.zeros(L, np.float32)
    lbl_cnts = np.zeros(L, np.float32)
    np.add.at(lbl_sums, labels, loss_j)
    np.add.at(lbl_cnts, labels, 1.0)
    loss_inter = -np.sum(
        np.where(lbl_cnts > 0, lbl_sums / np.maximum(lbl_cnts, 1.0), 0.0),
        dtype=np.float32)

    if int(epoch) < 5:
        return np.float32(loss_intra)
    return np.stack([loss_intra, LAMDA * loss_inter]).astype(np.float32)


def kernel(feats, centers, labels, camids, epoch):
    feats = np.ascontiguousarray(np.asarray(feats, dtype=np.float32))
    centers = np.ascontiguousarray(np.asarray(centers, dtype=np.float32))
    labels = np.asarray(labels).astype(np.int64)
    camids = np.asarray(camids).astype(np.int64)

    _order, counts = _sort_by_cam(camids)
    in_maps = _make_in_maps(feats, centers, camids)
    res = run_bass_kernel_spmd(_program(counts), in_maps,
                               list(range(NCORES))).results
    return _host_tail(res, feats, centers, labels, camids, epoch)
